# revision 30
# baseline (speedup 1.0000x reference)
"""NodeContrastiveLoss Trainium2 kernel, v3.

Full inputs -> scalar loss, data-parallel over 8 NeuronCores (256 batches/core).

Math per batch (reference semantics):
  sums[f,d] = segment-sum of atom_embed over atom2frag   (onehot.T @ atoms)
  mn        = sums/||sums||  (count scale cancels; clamp via +eps on ssq)
  sims      = 10 * mn @ fn_hat.T
  per_frag  = ln(sum_g exp(sims)) - sims[f,f]
  loss      = sum(valid*per_frag)/n_valid

v3 design (vs v1 baseline, 183us):
  - All heavy inputs quantized host-side to fp8 e3m4 and shipped as ONE
    uint8 blob DMA per iteration (448KB): atoms + host-built one-hot +
    fragT (fn_hat.T * 10, folding the 1/temperature).
  - Segment matmuls run TRANSPOSED (lhsT=atoms chunk [a,128d] -> FWL,
    out d-major [128d, b, 64f]), killing v1's DVE one-hot build, the
    diag(D) scale trick, and the transpose matmuls.
  - ssq and pos(=diag sims_raw) computed as elementwise-square / mult
    (DVE, 4x mode) + ones-vector matmuls (free-size-1, 2-batch merged
    lhsT [128,128] -> FWL) instead of TTR/skewed-DMA diag extraction.
  - s = (ssq+eps)^-0.5 in one DVE tensor_scalar (add, pow).
  - exp(s*G) via 4 ACT Exp ops with per-partition scale AP, PSUM->PSUM,
    per-pair; Sum_g via one DVE tensor_reduce into an s_sel slab.
  - Tail (ln, *mask, reduce) deferred to one slab-wide pass at the end.

Per-iter engines (est): DMA 1.25us, PE ~1.9us, DVE ~1.6us, ACT ~1.2us.
Host: sum 8x128 partials (negated), divide by n_valid.
"""

import sys

sys.path.insert(0, "/opt/trn_rl_repo")

from contextlib import ExitStack

import ml_dtypes
import numpy as np

import concourse.bacc as bacc
import concourse.bass as bass
import concourse.bass_utils as _bu
import concourse.tile as tile
from concourse import mybir
from concourse.bass_utils import run_bass_kernel_spmd

# NOTE: --enable-ldw-opt=true crashes walrus codegen (visitInstLdweights)
# on this BIR; the default false stays.

B, A, F_, D = 2048, 256, 64, 128
NCORES = 8
BPC = B // NCORES          # 256 batches per core
BPI = 8                    # batches per iteration
ITERS = BPC // BPI         # 32
PAIRS = BPI // 2           # 4
CHUNKS = BPI * 2           # 16 chunks of 128 atoms

BF16 = mybir.dt.bfloat16
F32 = mybir.dt.float32
U8 = mybir.dt.uint8
FP8 = mybir.dt.float8e3    # e3m4: 4 mantissa bits, range +-15.5
NP_FP8 = ml_dtypes.float8_e3m4
ALU = mybir.AluOpType
ACTF = mybir.ActivationFunctionType
AXIS = mybir.AxisListType

# blob layout per iter, bytes per partition
ATOMS_B = CHUNKS * 128     # 2048
OH_B = CHUNKS * 64         # 1024
FRAG_B = BPI * 64          # 512
BLOB_B = ATOMS_B + OH_B + FRAG_B  # 3584

EPS = 1e-12
# NOTE: tensor_tensor_reduce (custom DVE ucode) hangs the device under this
# runtime — stick to plain TT + tensor_reduce.
PROBE2_NO_ONES_MM = False
PROBE3_GUT_AFTER_COPY = False
PROBE4_NO_MATMUL = False


def build_body(tc, outs, ins):
    nc = tc.nc
    ctx = ExitStack()
    with ctx:
        const = ctx.enter_context(tc.tile_pool(name="const", bufs=1))
        dpool = ctx.enter_context(tc.tile_pool(name="dma", bufs=4))
        work = ctx.enter_context(tc.tile_pool(name="work", bufs=3))
        small = ctx.enter_context(tc.tile_pool(name="small", bufs=2))
        pseg = ctx.enter_context(tc.tile_pool(name="pseg", bufs=2, space="PSUM"))
        pg = ctx.enter_context(tc.tile_pool(name="pg", bufs=3, space="PSUM"))
        pe_ = ctx.enter_context(tc.tile_pool(name="pexp", bufs=2, space="PSUM"))
        psp = ctx.enter_context(tc.tile_pool(name="psp", bufs=1, space="PSUM"))

        # ---- one-time constants ----
        ones = const.tile([128, 1], BF16)
        nc.gpsimd.memset(ones[:], 1.0)
        eps_t = const.tile([128, 1], F32)
        nc.gpsimd.memset(eps_t[:], EPS)

        # slabs, fully written each run
        s_sel_slab = const.tile([128, ITERS, PAIRS], F32)
        s_slab = const.tile([128, ITERS, PAIRS], F32)

        mask_slab = const.tile([128, ITERS, PAIRS], BF16)
        nc.sync.dma_start(mask_slab[:], ins["mask"])

        # ssq/pos land in a persistent PSUM slab written by the ones-matmuls
        # (no per-iter pos copy op; tail reads pos straight from PSUM).
        psum_sp = psp.tile([128, 2, ITERS, PAIRS], F32)

        # Software-pipelined by one iteration. Per step i the PE program
        # order is [seg(i), G(i-1), sp(i-1)] — every matmul's inputs are
        # already computed when PE reaches it, so PE never head-of-line
        # stalls waiting on the DVE chain of the current iteration.
        stash = {}
        build_body._s2 = {}
        for it in range(ITERS + 2):
            if it < ITERS:
                # ---- stage A(it): DMA, seg matmuls, copy, sq, h ----
                # split the blob over the two independent DGE paths: atoms via
                # sync (HWDGE ring), onehot+fragT via gpsimd (SWDGE ring) —
                # the idle Pool engine eats the second dispatch.
                blob = dpool.tile([128, BLOB_B], U8, tag="blob")
                nc.sync.dma_start(blob[:, 0:ATOMS_B], ins["blob"][it][:, 0:ATOMS_B])
                nc.gpsimd.dma_start(
                    blob[:, ATOMS_B:BLOB_B], ins["blob"][it][:, ATOMS_B:BLOB_B]
                )
                atoms = (
                    blob[:, 0:ATOMS_B]
                    .bitcast(FP8)
                    .rearrange("p (c d) -> p c d", c=CHUNKS)
                )
                onehot = (
                    blob[:, ATOMS_B : ATOMS_B + OH_B]
                    .bitcast(FP8)
                    .rearrange("p (c f) -> p c f", c=CHUNKS)
                )
                fragT = (
                    blob[:, ATOMS_B + OH_B : BLOB_B]
                    .bitcast(FP8)
                    .rearrange("p (b g) -> p b g", b=BPI)
                )

                psum_segT = pseg.tile([128, BPI, 64], F32, tag="segT")
                for b in range(BPI):
                    for h in range(2):
                        c = b * 2 + h
                        nc.tensor.matmul(
                            psum_segT[:, b, :],
                            atoms[:, c, :],
                            onehot[:, c, :],
                            start=(h == 0),
                            stop=(h == 1),
                        )
                sumsT = work.tile([128, BPI, 64], BF16, tag="sumsT")
                nc.vector.tensor_copy(sumsT[:], psum_segT[:])

                sq = work.tile([128, BPI, 64], BF16, tag="sq")
                nc.vector.tensor_tensor(sq[:], sumsT[:], sumsT[:], op=ALU.mult)
                hprod = work.tile([128, BPI, 64], BF16, tag="h")
                nc.vector.tensor_tensor(hprod[:], sumsT[:], fragT, op=ALU.mult)
                stash[it] = (sumsT, fragT, sq, hprod)

            if it >= 1 and it - 1 < ITERS:
                # ---- stage B1(it-1): G + sp matmuls, s = rsqrt(ssq) ----
                ib = it - 1
                sumsT, fragT, sq, hprod = stash.pop(ib)

                psum_G = pg.tile([128, PAIRS, 64], F32, tag="G")
                for j in range(PAIRS):
                    for tw in range(2):
                        nc.tensor.matmul(
                            psum_G[64 * tw : 64 * tw + 64, j, :],
                            sumsT[:, 2 * j + tw, :],
                            fragT[:, 2 * j + tw, :],
                            start=True,
                            stop=True,
                        )

                for j in range(PAIRS):
                    nc.tensor.matmul(
                        psum_sp[:, 0, ib, j : j + 1],
                        sq[:, 2 * j : 2 * j + 2, :].rearrange("p b f -> p (b f)"),
                        ones[:],
                        start=True,
                        stop=True,
                    )
                    nc.tensor.matmul(
                        psum_sp[:, 1, ib, j : j + 1],
                        hprod[:, 2 * j : 2 * j + 2, :].rearrange(
                            "p b f -> p (b f)"
                        ),
                        ones[:],
                        start=True,
                        stop=True,
                    )

                lnssq = small.tile([128, PAIRS], F32, tag="lnssq")
                nc.scalar.activation(
                    lnssq[:], psum_sp[:, 0, ib, :], ACTF.Ln, bias=eps_t[:]
                )
                nc.scalar.activation(
                    s_slab[:, ib, :], lnssq[:], ACTF.Exp, scale=-0.5
                )
                stash2 = getattr(build_body, "_s2", {})
                stash2[ib] = psum_G
                build_body._s2 = stash2

            if it >= 2:
                # ---- stage B2(it-2): exp, s_sel — two iters behind the
                # G matmuls so the PE->ACT semaphores are long satisfied ----
                ic = it - 2
                psum_G = build_body._s2.pop(ic)

                psum_exp = pe_.tile([128, PAIRS, 64], F32, tag="exp")
                for j in range(PAIRS):
                    nc.scalar.activation(
                        psum_exp[:, j, :],
                        psum_G[:, j, :],
                        ACTF.Exp,
                        scale=s_slab[:, ic, j : j + 1],
                    )

                nc.vector.tensor_reduce(
                    s_sel_slab[:, ic, :], psum_exp[:], axis=AXIS.X, op=ALU.add
                )

        # ---- tail: -per_frag = s*pos - ln(s_sel); masked total ----
        ln_sel = const.tile([128, ITERS, PAIRS], F32)
        nc.scalar.activation(ln_sel[:], s_sel_slab[:], ACTF.Ln)
        # neg_pf = (pos * s) - ln_sel  (pos read straight from the PSUM slab)
        neg_pf = const.tile([128, ITERS, PAIRS], F32)
        spos = const.tile([128, ITERS, PAIRS], F32)
        nc.vector.tensor_tensor(spos[:], psum_sp[:, 1, :, :], s_slab[:], op=ALU.mult)
        nc.vector.tensor_tensor(neg_pf[:], spos[:], ln_sel[:], op=ALU.subtract)
        junk = const.tile([128, ITERS, PAIRS], F32)
        outsb = const.tile([128, 1], F32)
        nc.vector.tensor_tensor(junk[:], neg_pf[:], mask_slab[:], op=ALU.mult)
        nc.vector.tensor_reduce(outsb[:], junk[:], axis=AXIS.XY, op=ALU.add)
        nc.sync.dma_start(outs["out"], outsb[:])


def prep_inputs(atom_embed, fragment_embed, atom2frag):
    """Host-side layout prep. Returns (in_maps, n_valid)."""
    am = np.asarray(atom_embed, dtype=np.float32)
    fe = np.asarray(fragment_embed, dtype=np.float32)
    af = np.asarray(atom2frag)

    # atoms fp8: [B,A,D] -> [core, it, p(=a%128), c(=b*2+h), d] bytes
    a5 = am.reshape(NCORES, ITERS, BPI, 2, 128, D)  # [nc, it, b, h, p, d]
    atoms8 = np.ascontiguousarray(
        a5.transpose(0, 1, 4, 2, 3, 5)
    ).astype(NP_FP8)  # [nc, it, p, b, h, d]
    atoms_bytes = atoms8.reshape(NCORES, ITERS, 128, ATOMS_B).view(np.uint8)

    # one-hot fp8: oh[nc, it, p, b, h, f] = (af_chunk[p] == f)
    i5 = af.reshape(NCORES, ITERS, BPI, 2, 128).transpose(0, 1, 4, 2, 3)
    # i5: [nc, it, p, b, h]
    oh = (i5[..., None] == np.arange(F_)[None, None, None, None, None, :]).astype(
        NP_FP8
    )
    oh_bytes = oh.reshape(NCORES, ITERS, 128, OH_B).view(np.uint8)

    # fragT fp8 (fn_hat.T * 10): [nc, it, p(=d), b, g]
    fen = 10.0 * fe / np.maximum(np.linalg.norm(fe, axis=-1, keepdims=True), 1e-8)
    f4 = fen.reshape(NCORES, ITERS, BPI, F_, D)
    fragT8 = np.ascontiguousarray(f4.transpose(0, 1, 4, 2, 3)).astype(NP_FP8)
    frag_bytes = fragT8.reshape(NCORES, ITERS, 128, FRAG_B).view(np.uint8)

    blob = np.concatenate([atoms_bytes, oh_bytes, frag_bytes], axis=-1)
    assert blob.shape == (NCORES, ITERS, 128, BLOB_B)

    # mask: [nc, p(=tw*64+f), it, j]
    counts = (af[:, :, None] == np.arange(F_)[None, None, :]).sum(axis=1)
    valid = counts > 0
    n_valid = int(valid.sum())
    v5 = valid.reshape(NCORES, ITERS, PAIRS, 2, F_)  # [nc, it, j, tw, f]
    mask_np = np.ascontiguousarray(
        v5.transpose(0, 3, 4, 1, 2).reshape(NCORES, 128, ITERS, PAIRS)
    ).astype(ml_dtypes.bfloat16)

    in_maps = [
        {"blob": blob[k], "mask": mask_np[k]}
        for k in range(NCORES)
    ]
    return in_maps, n_valid


_BUILT = None


def build_nc():
    global _BUILT
    if _BUILT is not None:
        return _BUILT
    nc = bacc.Bacc("TRN2", target_bir_lowering=False, debug=False)
    ins = {
        "blob": nc.dram_tensor(
            "blob", [ITERS, 128, BLOB_B], U8, kind="ExternalInput"
        ).ap(),
        "mask": nc.dram_tensor(
            "mask", [128, ITERS, PAIRS], BF16, kind="ExternalInput"
        ).ap(),
    }
    outs = {"out": nc.dram_tensor("out", [128, 1], F32, kind="ExternalOutput").ap()}
    with tile.TileContext(nc) as tc:
        build_body(tc, outs, ins)
    nc.compile()
    _fix_act_table_loads(nc)
    _BUILT = nc
    return nc


def _fix_act_table_loads(nc):
    """Collapse act-table loads into one load of natural_log_exp_and_others
    (serves Copy/Ln/Exp), saving ~1.3us per extra load."""
    from concourse.hw_specs import get_activation_tables

    tables = list(get_activation_tables(nc.m.arch).keys())
    target = tables.index("natural_log_exp_and_others")
    kept = False
    for f in nc.m.functions:
        for b in f.blocks:
            keep = []
            for i in b.instructions:
                if isinstance(i, mybir.InstLoadActFuncSet):
                    si = i.sync_info
                    assert si is None or (not si.on_wait and not si.on_update)
                    if kept:
                        continue
                    i.act_func_set_id = target
                    kept = True
                keep.append(i)
            b.instructions[:] = keep


def run_on_hw(in_maps, trace=False, **kw):
    nc = build_nc()
    return run_bass_kernel_spmd(nc, in_maps, list(range(NCORES)), trace=trace, **kw)


def kernel(**inputs) -> np.ndarray:
    in_maps, n_valid = prep_inputs(
        inputs["atom_embed"], inputs["fragment_embed"], inputs["atom2frag"]
    )
    res = run_on_hw(in_maps)
    total = 0.0
    for k in range(NCORES):
        # device accumulated (s*pos - ln(s_sel)) * mask = -sum(per_frag)
        total -= float(np.asarray(res.results[k]["out"], dtype=np.float64).sum())
    if n_valid > 0:
        loss = np.float32(total / n_valid)
    else:
        loss = np.float32(0.0)
    return np.array(loss, dtype=np.float32)


# revision 35
# speedup vs baseline: 1.0447x; 1.0447x over previous
"""NodeContrastiveLoss Trainium2 kernel, v3.

Full inputs -> scalar loss, data-parallel over 8 NeuronCores (256 batches/core).

Math per batch (reference semantics):
  sums[f,d] = segment-sum of atom_embed over atom2frag   (onehot.T @ atoms)
  mn        = sums/||sums||  (count scale cancels; clamp via +eps on ssq)
  sims      = 10 * mn @ fn_hat.T
  per_frag  = ln(sum_g exp(sims)) - sims[f,f]
  loss      = sum(valid*per_frag)/n_valid

v3 design (vs v1 baseline, 183us):
  - All heavy inputs quantized host-side to fp8 e3m4 and shipped as ONE
    uint8 blob DMA per iteration (448KB): atoms + host-built one-hot +
    fragT (fn_hat.T * 10, folding the 1/temperature).
  - Segment matmuls run TRANSPOSED (lhsT=atoms chunk [a,128d] -> FWL,
    out d-major [128d, b, 64f]), killing v1's DVE one-hot build, the
    diag(D) scale trick, and the transpose matmuls.
  - ssq and pos(=diag sims_raw) computed as elementwise-square / mult
    (DVE, 4x mode) + ones-vector matmuls (free-size-1, 2-batch merged
    lhsT [128,128] -> FWL) instead of TTR/skewed-DMA diag extraction.
  - s = (ssq+eps)^-0.5 in one DVE tensor_scalar (add, pow).
  - exp(s*G) via 4 ACT Exp ops with per-partition scale AP, PSUM->PSUM,
    per-pair; Sum_g via one DVE tensor_reduce into an s_sel slab.
  - Tail (ln, *mask, reduce) deferred to one slab-wide pass at the end.

Per-iter engines (est): DMA 1.25us, PE ~1.9us, DVE ~1.6us, ACT ~1.2us.
Host: sum 8x128 partials (negated), divide by n_valid.
"""

import sys

sys.path.insert(0, "/opt/trn_rl_repo")

from contextlib import ExitStack

import ml_dtypes
import numpy as np

import concourse.bacc as bacc
import concourse.bass as bass
import concourse.bass_utils as _bu
import concourse.tile as tile
from concourse import mybir
from concourse.bass_utils import run_bass_kernel_spmd

# NOTE: --enable-ldw-opt=true crashes walrus codegen (visitInstLdweights)
# on this BIR; the default false stays.

B, A, F_, D = 2048, 256, 64, 128
NCORES = 8
BPC = B // NCORES          # 256 batches per core
BPI = 8                    # batches per iteration
ITERS = BPC // BPI         # 32
PAIRS = BPI // 2           # 4
CHUNKS = BPI * 2           # 16 chunks of 128 atoms

BF16 = mybir.dt.bfloat16
F32 = mybir.dt.float32
U8 = mybir.dt.uint8
FP8 = mybir.dt.float8e3    # e3m4: 4 mantissa bits, range +-15.5
NP_FP8 = ml_dtypes.float8_e3m4
ALU = mybir.AluOpType
ACTF = mybir.ActivationFunctionType
AXIS = mybir.AxisListType

# blob layout per iter, bytes per partition
ATOMS_B = CHUNKS * 128     # 2048
OH_B = CHUNKS * 64         # 1024
FRAG_B = BPI * 64          # 512
BLOB_B = ATOMS_B + OH_B + FRAG_B  # 3584

EPS = 1e-12
# NOTE: tensor_tensor_reduce (custom DVE ucode) hangs the device under this
# runtime — stick to plain TT + tensor_reduce.
PROBE2_NO_ONES_MM = False
PROBE3_GUT_AFTER_COPY = False
PROBE4_NO_MATMUL = False


def build_body(tc, outs, ins):
    nc = tc.nc
    ctx = ExitStack()
    with ctx:
        const = ctx.enter_context(tc.tile_pool(name="const", bufs=1))
        dpool = ctx.enter_context(tc.tile_pool(name="dma", bufs=4))
        work = ctx.enter_context(tc.tile_pool(name="work", bufs=3))
        small = ctx.enter_context(tc.tile_pool(name="small", bufs=2))
        pseg = ctx.enter_context(tc.tile_pool(name="pseg", bufs=2, space="PSUM"))
        pg = ctx.enter_context(tc.tile_pool(name="pg", bufs=3, space="PSUM"))
        pe_ = ctx.enter_context(tc.tile_pool(name="pexp", bufs=2, space="PSUM"))
        psp = ctx.enter_context(tc.tile_pool(name="psp", bufs=1, space="PSUM"))

        # ---- one-time constants ----
        ones = const.tile([128, 1], BF16)
        nc.gpsimd.memset(ones[:], 1.0)
        eps_t = const.tile([128, 1], F32)
        nc.gpsimd.memset(eps_t[:], EPS)

        # slabs, fully written each run
        s_half_slab = const.tile([128, ITERS, PAIRS, 2], F32)
        s_slab = const.tile([128, ITERS, PAIRS], F32)

        mask_slab = const.tile([128, ITERS, PAIRS], BF16)
        nc.sync.dma_start(mask_slab[:], ins["mask"])

        # ssq/pos land in a persistent PSUM slab written by the ones-matmuls
        # (no per-iter pos copy op; tail reads pos straight from PSUM).
        psum_sp = psp.tile([128, 2, ITERS, PAIRS], F32)

        # Software-pipelined by one iteration. Per step i the PE program
        # order is [seg(i), G(i-1), sp(i-1)] — every matmul's inputs are
        # already computed when PE reaches it, so PE never head-of-line
        # stalls waiting on the DVE chain of the current iteration.
        stash = {}
        build_body._s2 = {}
        for it in range(ITERS + 2):
            if it < ITERS:
                # ---- stage A(it): DMA, seg matmuls, copy, sq, h ----
                blob = dpool.tile([128, BLOB_B], U8, tag="blob")
                nc.sync.dma_start(blob[:], ins["blob"][it])
                atoms = (
                    blob[:, 0:ATOMS_B]
                    .bitcast(FP8)
                    .rearrange("p (c d) -> p c d", c=CHUNKS)
                )
                onehot = (
                    blob[:, ATOMS_B : ATOMS_B + OH_B]
                    .bitcast(FP8)
                    .rearrange("p (c f) -> p c f", c=CHUNKS)
                )
                fragT = (
                    blob[:, ATOMS_B + OH_B : BLOB_B]
                    .bitcast(FP8)
                    .rearrange("p (b g) -> p b g", b=BPI)
                )

                psum_segT = pseg.tile([128, BPI, 64], F32, tag="segT")
                for b in range(BPI):
                    for h in range(2):
                        c = b * 2 + h
                        nc.tensor.matmul(
                            psum_segT[:, b, :],
                            atoms[:, c, :],
                            onehot[:, c, :],
                            start=(h == 0),
                            stop=(h == 1),
                        )
                sumsT = work.tile([128, BPI, 64], BF16, tag="sumsT")
                nc.vector.tensor_copy(sumsT[:], psum_segT[:])

                sq = work.tile([128, BPI, 64], BF16, tag="sq")
                nc.vector.tensor_tensor(sq[:], sumsT[:], sumsT[:], op=ALU.mult)
                hprod = work.tile([128, BPI, 64], BF16, tag="h")
                nc.vector.tensor_tensor(hprod[:], sumsT[:], fragT, op=ALU.mult)
                stash[it] = (sumsT, fragT, sq, hprod)

            if it >= 1 and it - 1 < ITERS:
                # ---- stage B1(it-1): G + sp matmuls, s = rsqrt(ssq) ----
                ib = it - 1
                sumsT, fragT, sq, hprod = stash.pop(ib)

                # merged G: lhsT = both batches of the pair [128,128], rhs =
                # both batches' fragT [128,128]; the off-diagonal 64x64
                # blocks are cross-batch garbage, dropped at half-select.
                psum_G = pg.tile([128, PAIRS, 128], F32, tag="G")
                for j in range(PAIRS):
                    nc.tensor.matmul(
                        psum_G[:, j, :],
                        sumsT[:, 2 * j : 2 * j + 2, :].rearrange(
                            "p b f -> p (b f)"
                        ),
                        fragT[:, 2 * j : 2 * j + 2, :].rearrange(
                            "p b g -> p (b g)"
                        ),
                        start=True,
                        stop=True,
                    )

                for j in range(PAIRS):
                    nc.tensor.matmul(
                        psum_sp[:, 0, ib, j : j + 1],
                        sq[:, 2 * j : 2 * j + 2, :].rearrange("p b f -> p (b f)"),
                        ones[:],
                        start=True,
                        stop=True,
                    )
                    nc.tensor.matmul(
                        psum_sp[:, 1, ib, j : j + 1],
                        hprod[:, 2 * j : 2 * j + 2, :].rearrange(
                            "p b f -> p (b f)"
                        ),
                        ones[:],
                        start=True,
                        stop=True,
                    )

                lnssq = small.tile([128, PAIRS], F32, tag="lnssq")
                nc.scalar.activation(
                    lnssq[:], psum_sp[:, 0, ib, :], ACTF.Ln, bias=eps_t[:]
                )
                nc.scalar.activation(
                    s_slab[:, ib, :], lnssq[:], ACTF.Exp, scale=-0.5
                )
                stash2 = getattr(build_body, "_s2", {})
                stash2[ib] = psum_G
                build_body._s2 = stash2

            if it >= 2:
                # ---- stage B2(it-2): exp, s_sel — two iters behind the
                # G matmuls so the PE->ACT semaphores are long satisfied ----
                ic = it - 2
                psum_G = build_body._s2.pop(ic)

                psum_exp = pe_.tile([128, PAIRS, 128], F32, tag="exp")
                for j in range(PAIRS):
                    nc.scalar.activation(
                        psum_exp[:, j, :],
                        psum_G[:, j, :],
                        ACTF.Exp,
                        scale=s_slab[:, ic, j : j + 1],
                    )

                # per-half sums; own-half selection deferred to the tail
                nc.vector.tensor_reduce(
                    s_half_slab[:, ic, :, :],
                    psum_exp[:].rearrange("p j (t g) -> p j t g", t=2),
                    axis=AXIS.X,
                    op=ALU.add,
                )

        # ---- tail: -per_frag = s*pos - ln(s_sel); masked total ----
        # select own half: partitions 0:64 summed over g-cols 0:64, etc.
        s_sel_slab = const.tile([128, ITERS, PAIRS], F32)
        nc.vector.tensor_copy(s_sel_slab[0:64, :, :], s_half_slab[0:64, :, :, 0])
        nc.vector.tensor_copy(s_sel_slab[64:128, :, :], s_half_slab[64:128, :, :, 1])
        ln_sel = const.tile([128, ITERS, PAIRS], F32)
        nc.scalar.activation(ln_sel[:], s_sel_slab[:], ACTF.Ln)
        # neg_pf = (pos * s) - ln_sel  (pos read straight from the PSUM slab)
        neg_pf = const.tile([128, ITERS, PAIRS], F32)
        spos = const.tile([128, ITERS, PAIRS], F32)
        nc.vector.tensor_tensor(spos[:], psum_sp[:, 1, :, :], s_slab[:], op=ALU.mult)
        nc.vector.tensor_tensor(neg_pf[:], spos[:], ln_sel[:], op=ALU.subtract)
        junk = const.tile([128, ITERS, PAIRS], F32)
        outsb = const.tile([128, 1], F32)
        nc.vector.tensor_tensor(junk[:], neg_pf[:], mask_slab[:], op=ALU.mult)
        nc.vector.tensor_reduce(outsb[:], junk[:], axis=AXIS.XY, op=ALU.add)
        nc.sync.dma_start(outs["out"], outsb[:])


def prep_inputs(atom_embed, fragment_embed, atom2frag):
    """Host-side layout prep. Returns (in_maps, n_valid)."""
    am = np.asarray(atom_embed, dtype=np.float32)
    fe = np.asarray(fragment_embed, dtype=np.float32)
    af = np.asarray(atom2frag)

    # atoms fp8: [B,A,D] -> [core, it, p(=a%128), c(=b*2+h), d] bytes
    a5 = am.reshape(NCORES, ITERS, BPI, 2, 128, D)  # [nc, it, b, h, p, d]
    atoms8 = np.ascontiguousarray(
        a5.transpose(0, 1, 4, 2, 3, 5)
    ).astype(NP_FP8)  # [nc, it, p, b, h, d]
    atoms_bytes = atoms8.reshape(NCORES, ITERS, 128, ATOMS_B).view(np.uint8)

    # one-hot fp8: oh[nc, it, p, b, h, f] = (af_chunk[p] == f)
    i5 = af.reshape(NCORES, ITERS, BPI, 2, 128).transpose(0, 1, 4, 2, 3)
    # i5: [nc, it, p, b, h]
    oh = (i5[..., None] == np.arange(F_)[None, None, None, None, None, :]).astype(
        NP_FP8
    )
    oh_bytes = oh.reshape(NCORES, ITERS, 128, OH_B).view(np.uint8)

    # fragT fp8 (fn_hat.T * 10): [nc, it, p(=d), b, g]
    fen = 10.0 * fe / np.maximum(np.linalg.norm(fe, axis=-1, keepdims=True), 1e-8)
    f4 = fen.reshape(NCORES, ITERS, BPI, F_, D)
    fragT8 = np.ascontiguousarray(f4.transpose(0, 1, 4, 2, 3)).astype(NP_FP8)
    frag_bytes = fragT8.reshape(NCORES, ITERS, 128, FRAG_B).view(np.uint8)

    blob = np.concatenate([atoms_bytes, oh_bytes, frag_bytes], axis=-1)
    assert blob.shape == (NCORES, ITERS, 128, BLOB_B)

    # mask: [nc, p(=tw*64+f), it, j]
    counts = (af[:, :, None] == np.arange(F_)[None, None, :]).sum(axis=1)
    valid = counts > 0
    n_valid = int(valid.sum())
    v5 = valid.reshape(NCORES, ITERS, PAIRS, 2, F_)  # [nc, it, j, tw, f]
    mask_np = np.ascontiguousarray(
        v5.transpose(0, 3, 4, 1, 2).reshape(NCORES, 128, ITERS, PAIRS)
    ).astype(ml_dtypes.bfloat16)

    in_maps = [
        {"blob": blob[k], "mask": mask_np[k]}
        for k in range(NCORES)
    ]
    return in_maps, n_valid


_BUILT = None


def build_nc():
    global _BUILT
    if _BUILT is not None:
        return _BUILT
    nc = bacc.Bacc("TRN2", target_bir_lowering=False, debug=False)
    ins = {
        "blob": nc.dram_tensor(
            "blob", [ITERS, 128, BLOB_B], U8, kind="ExternalInput"
        ).ap(),
        "mask": nc.dram_tensor(
            "mask", [128, ITERS, PAIRS], BF16, kind="ExternalInput"
        ).ap(),
    }
    outs = {"out": nc.dram_tensor("out", [128, 1], F32, kind="ExternalOutput").ap()}
    with tile.TileContext(nc) as tc:
        build_body(tc, outs, ins)
    nc.compile()
    _fix_act_table_loads(nc)
    _BUILT = nc
    return nc


def _fix_act_table_loads(nc):
    """Collapse act-table loads into one load of natural_log_exp_and_others
    (serves Copy/Ln/Exp), saving ~1.3us per extra load."""
    from concourse.hw_specs import get_activation_tables

    tables = list(get_activation_tables(nc.m.arch).keys())
    target = tables.index("natural_log_exp_and_others")
    kept = False
    for f in nc.m.functions:
        for b in f.blocks:
            keep = []
            for i in b.instructions:
                if isinstance(i, mybir.InstLoadActFuncSet):
                    si = i.sync_info
                    assert si is None or (not si.on_wait and not si.on_update)
                    if kept:
                        continue
                    i.act_func_set_id = target
                    kept = True
                keep.append(i)
            b.instructions[:] = keep


def run_on_hw(in_maps, trace=False, **kw):
    nc = build_nc()
    return run_bass_kernel_spmd(nc, in_maps, list(range(NCORES)), trace=trace, **kw)


def kernel(**inputs) -> np.ndarray:
    in_maps, n_valid = prep_inputs(
        inputs["atom_embed"], inputs["fragment_embed"], inputs["atom2frag"]
    )
    res = run_on_hw(in_maps)
    total = 0.0
    for k in range(NCORES):
        # device accumulated (s*pos - ln(s_sel)) * mask = -sum(per_frag)
        total -= float(np.asarray(res.results[k]["out"], dtype=np.float64).sum())
    if n_valid > 0:
        loss = np.float32(total / n_valid)
    else:
        loss = np.float32(0.0)
    return np.array(loss, dtype=np.float32)
